# revision 8
# baseline (speedup 1.0000x reference)
"""MoE layer (8 routed experts top-2 + 2 shared experts) on 8 Trainium2 cores.

Strategy: expert-parallel, dense-masked.
  - Core c holds routed expert c's weights plus a 1/8 I-slice of both shared
    experts, concatenated into one augmented SwiGLU FFN of width 1280.
  - X is replicated (host sends X.T so every matmul is transpose-free:
    contraction dim H/I always lands on SBUF partitions).
  - Router: each core computes fp32 gate logits for its T/8 token shard on
    the PE, AllGather -> identical full logits everywhere; top-2 / softmax /
    combine weights / aux loss computed with DVE/ACT ops.
  - FFN matmuls run in float32r (full PE rate at N>=256, ~1.5e-4 rel err).
  - Tokens are processed in pairs of 512-token chunks; one pass over the
    w1/w2/w3 weights covers both chunks of a pair (halves weight traffic).
  - Routed output scaled per-token by the combine weight at PSUM drain
    (ACT Copy with per-partition scale); shared output added with a 0.5
    factor (SH=2 mean) in the same fused DVE op.
  - Output combined across cores with a per-512-token-chunk ReduceScatter
    (overlaps with compute); host reassembles the shards.

kernel(**inputs) -> (out [2,1024,2048] fp32, router_loss fp32 scalar),
matching reference.reference()'s return structure.
"""

import numpy as np

# ---- problem constants (hardcoded; kernel.py must be self-contained) ----
B, S, H, I = 2, 1024, 2048, 1024
E, K_TOP, SH = 8, 2, 2
AUX_COEF = 0.01
T = B * S                  # 2048 tokens
NC_ = 8                    # cores
ISH = I // NC_             # 128: per-core slice of each shared expert
IAUG = I + SH * ISH        # 1280 augmented intermediate width
TC = 512                   # token chunk
NCHUNK = T // TC           # 4
TSH = T // NC_             # 256: tokens per core shard (router + output)
KT = H // 128              # 16 contraction tiles
MT = IAUG // 128           # 10 I tiles
MR = I // 128              # 8 routed I tiles
HC = 256                   # H chunk in stage 2
NH = H // HC               # 8

_PROG = None  # cached compiled program


def _build():
    import concourse.bacc as bacc
    import concourse.mybir as mybir
    import concourse.tile as tile

    f32 = mybir.dt.float32
    f32r = mybir.dt.float32r
    Alu = mybir.AluOpType
    Act = mybir.ActivationFunctionType
    RG = [list(range(NC_))]

    nc = bacc.Bacc("TRN2", target_bir_lowering=False, debug=False, num_devices=NC_)

    xT_ap = nc.dram_tensor("xT", [H, T], f32r, kind="ExternalInput").ap()
    xsh_ap = nc.dram_tensor("xsh", [H, TSH], f32, kind="ExternalInput").ap()
    gw_ap = nc.dram_tensor("gw", [H, E], f32, kind="ExternalInput").ap()
    esel_ap = nc.dram_tensor("esel", [128, E], f32, kind="ExternalInput").ap()
    # w1/w2 blocked: [MT, 128p, KT*128] with [m, p, k*128+j] = w1[k*128+p, m*128+j]
    w1_ap = nc.dram_tensor("w1", [MT, 128, KT * 128], f32r, kind="ExternalInput").ap()
    w2_ap = nc.dram_tensor("w2", [MT, 128, KT * 128], f32r, kind="ExternalInput").ap()
    # w3 blocked: [NH, 128p, MT*HC] with [nh, p, kI*HC+j] = w3[kI*128+p, nh*HC+j]
    w3_ap = nc.dram_tensor("w3", [NH, 128, MT * HC], f32r, kind="ExternalInput").ap()
    out_ap = nc.dram_tensor("out_sh", [NCHUNK, TC // NC_, H], f32, kind="ExternalOutput").ap()
    loss_ap = nc.dram_tensor("loss", [1, 1], f32, kind="ExternalOutput").ap()

    TI = T // 128  # 16

    with tile.TileContext(nc) as tc:
      with (
        tc.tile_pool(name="sbr", bufs=1) as sbr,
        tc.tile_pool(name="dram", bufs=1, space="DRAM") as dram,
        tc.tile_pool(name="xp", bufs=2) as x_pool,
        tc.tile_pool(name="wblk", bufs=3) as wblk_pool,
      ):
        # pre-emit pair-0 x and first weight-block DMAs so HBM streaming
        # starts immediately (PE warms up on stage-1 without waiting)
        x_tiles = {}
        for n in range(2):
            x_sb = x_pool.tile([128, KT, TC], f32r, name=f"x{n}", tag="x")
            for k in range(KT):
                nc.sync.dma_start(
                    out=x_sb[:, k, :],
                    in_=xT_ap[k * 128:(k + 1) * 128, n * TC:(n + 1) * TC],
                )
            x_tiles[n] = x_sb

        w_tiles = {}

        def fetch_wblk(pair, m):
            if (pair, m) in w_tiles:
                return w_tiles.pop((pair, m))
            w1b = wblk_pool.tile([128, KT, 128], f32r, name=f"w1_{pair}_{m}", tag="w1")
            nc.sync.dma_start(out=w1b[:], in_=w1_ap[m].rearrange("p (k j) -> p k j", k=KT))
            w2b = wblk_pool.tile([128, KT, 128], f32r, name=f"w2_{pair}_{m}", tag="w2")
            nc.sync.dma_start(out=w2b[:], in_=w2_ap[m].rearrange("p (k j) -> p k j", k=KT))
            return w1b, w2b

        for m in range(2):
            w1b = wblk_pool.tile([128, KT, 128], f32r, name=f"w1_0_{m}", tag="w1")
            nc.sync.dma_start(out=w1b[:], in_=w1_ap[m].rearrange("p (k j) -> p k j", k=KT))
            w2b = wblk_pool.tile([128, KT, 128], f32r, name=f"w2_0_{m}", tag="w2")
            nc.sync.dma_start(out=w2b[:], in_=w2_ap[m].rearrange("p (k j) -> p k j", k=KT))
            w_tiles[(0, m)] = (w1b, w2b)

        # ---------------- phase A: gate logits for own shard + AllGather --
        with (
            tc.tile_pool(name="xshp", bufs=1) as xsh_pool,
            tc.tile_pool(name="psLG", bufs=2, space="PSUM") as psLG,
            tc.tile_pool(name="psL", bufs=1, space="PSUM") as psL,
        ):
            xsh_sb = xsh_pool.tile([128, KT, TSH], f32, name="xsh_sb")
            for k in range(KT):
                nc.sync.dma_start(out=xsh_sb[:, k, :], in_=xsh_ap[k * 128:(k + 1) * 128, :])
            gw_sb = sbr.tile([128, KT, E], f32, name="gw_sb")
            for k in range(KT):
                nc.sync.dma_start(out=gw_sb[:, k, :], in_=gw_ap[k * 128:(k + 1) * 128, :])
            lgsh = sbr.tile([128, TSH // 128, E], f32, name="lgsh")
            for tt in range(TSH // 128):
                plg = psLG.tile([128, E], f32, name=f"plg{tt}", tag="plg")
                for k in range(KT):
                    nc.tensor.matmul(
                        plg[:], xsh_sb[:, k, tt * 128:(tt + 1) * 128], gw_sb[:, k, :],
                        start=(k == 0), stop=(k == KT - 1),
                    )
                nc.vector.tensor_copy(lgsh[:, tt, :], plg[:])
            ag_in = dram.tile([TSH, E], f32, name="ag_in")
            nc.sync.dma_start(
                out=ag_in[:].rearrange("(tt p) e -> p tt e", p=128), in_=lgsh[:]
            )
            lg_full = dram.tile([T, E], f32, name="lg_full", addr_space="Shared")
            nc.gpsimd.collective_compute(
                "AllGather", Alu.bypass, replica_groups=RG,
                ins=[ag_in.opt()], outs=[lg_full.opt()],
            )
            lg = sbr.tile([128, TI, E], f32, name="lg")
            nc.sync.dma_start(out=lg[:], in_=lg_full[:].rearrange("(ti p) e -> p ti e", p=128))

            # ------------- phase B: router math (identical on all cores) --
            def b3(ap2d):  # [128, TI] -> broadcast [128, TI, E]
                return ap2d.rearrange("p (t o) -> p t o", o=1).broadcast_to([128, TI, E])

            m1 = sbr.tile([128, TI], f32, name="m1")
            nc.vector.tensor_reduce(m1[:], lg[:], axis=mybir.AxisListType.X, op=Alu.max)
            mask = sbr.tile([128, TI, E], f32, name="mask")
            nc.vector.tensor_tensor(mask[:], lg[:], b3(m1[:]), op=Alu.is_equal)
            l2 = sbr.tile([128, TI, E], f32, name="l2")
            nc.vector.scalar_tensor_tensor(l2[:], mask[:], -1e30, lg[:], op0=Alu.mult, op1=Alu.add)
            m2 = sbr.tile([128, TI], f32, name="m2")
            nc.vector.tensor_reduce(m2[:], l2[:], axis=mybir.AxisListType.X, op=Alu.max)

            dlg = sbr.tile([128, TI, E], f32, name="dlg")
            nc.vector.tensor_tensor(dlg[:], lg[:], b3(m1[:]), op=Alu.subtract)
            pex = sbr.tile([128, TI, E], f32, name="pex")
            nc.scalar.activation(pex[:], dlg[:], Act.Exp)
            Z = sbr.tile([128, TI], f32, name="Z")
            nc.vector.tensor_reduce(Z[:], pex[:], axis=mybir.AxisListType.X, op=Alu.add)
            rZ = sbr.tile([128, TI], f32, name="rZ")
            nc.vector.reciprocal(rZ[:], Z[:])
            pn = sbr.tile([128, TI, E], f32, name="pn")
            nc.vector.tensor_tensor(pn[:], pex[:], b3(rZ[:]), op=Alu.mult)
            ind2 = sbr.tile([128, TI, E], f32, name="ind2")
            nc.vector.tensor_tensor(ind2[:], lg[:], b3(m2[:]), op=Alu.is_ge)

            sum_p = sbr.tile([128, E], f32, name="sum_p")
            nc.vector.tensor_reduce(
                sum_p[:], pn[:].rearrange("p t e -> p e t"),
                axis=mybir.AxisListType.X, op=Alu.add,
            )
            sum_i = sbr.tile([128, E], f32, name="sum_i")
            nc.vector.tensor_reduce(
                sum_i[:], ind2[:].rearrange("p t e -> p e t"),
                axis=mybir.AxisListType.X, op=Alu.add,
            )
            ones = sbr.tile([128, 1], f32, name="ones")
            nc.vector.memset(ones[:], 1.0)
            ptpe = psL.tile([1, E], f32, name="ptpe", tag="ptpe")
            nc.tensor.matmul(ptpe[:], ones[:], sum_i[:], start=True, stop=True)
            pppe = psL.tile([1, E], f32, name="pppe", tag="pppe")
            nc.tensor.matmul(pppe[:], ones[:], sum_p[:], start=True, stop=True)
            ppe_sb = sbr.tile([1, E], f32, name="ppe_sb")
            nc.vector.tensor_copy(ppe_sb[:], pppe[:])
            lt = sbr.tile([1, E], f32, name="lt")
            nc.vector.tensor_tensor(lt[:], ptpe[:], ppe_sb[:], op=Alu.mult)
            ls = sbr.tile([1, 1], f32, name="ls")
            nc.vector.tensor_reduce(ls[:], lt[:], axis=mybir.AxisListType.X, op=Alu.add)
            lsc = sbr.tile([1, 1], f32, name="lsc")
            nc.vector.tensor_scalar_mul(lsc[:], ls[:], float(E) * AUX_COEF / (T * T))
            nc.sync.dma_start(out=loss_ap[:], in_=lsc[:])

            # combine weight for this core's expert: w = exp(lc-m1)*1{lc>=m2}/(1+exp(m2-m1))
            esel_sb = sbr.tile([128, E], f32, name="esel_sb")
            nc.sync.dma_start(out=esel_sb[:], in_=esel_ap[:])
            eselb = esel_sb[:].rearrange("p (t e) -> p t e", t=1).broadcast_to([128, TI, E])
            lcq = sbr.tile([128, TI, E], f32, name="lcq")
            nc.vector.tensor_tensor(lcq[:], lg[:], eselb, op=Alu.mult)
            lc = sbr.tile([128, TI], f32, name="lc")
            nc.vector.tensor_reduce(lc[:], lcq[:], axis=mybir.AxisListType.X, op=Alu.add)
            d2 = sbr.tile([128, TI], f32, name="d2")
            nc.vector.tensor_tensor(d2[:], lc[:], m1[:], op=Alu.subtract)
            num = sbr.tile([128, TI], f32, name="num")
            nc.scalar.activation(num[:], d2[:], Act.Exp)
            d3 = sbr.tile([128, TI], f32, name="d3")
            nc.vector.tensor_tensor(d3[:], m2[:], m1[:], op=Alu.subtract)
            a2 = sbr.tile([128, TI], f32, name="a2")
            nc.scalar.activation(a2[:], d3[:], Act.Exp)
            den = sbr.tile([128, TI], f32, name="den")
            nc.vector.tensor_scalar_add(den[:], a2[:], 1.0)
            rden = sbr.tile([128, TI], f32, name="rden")
            nc.vector.reciprocal(rden[:], den[:])
            ind = sbr.tile([128, TI], f32, name="ind")
            nc.vector.tensor_tensor(ind[:], lc[:], m2[:], op=Alu.is_ge)
            wA = sbr.tile([128, TI], f32, name="wA")
            nc.vector.tensor_tensor(wA[:], num[:], ind[:], op=Alu.mult)
            w_sb = sbr.tile([128, TI], f32, name="w_sb")
            nc.vector.tensor_tensor(w_sb[:], wA[:], rden[:], op=Alu.mult)

        # ---------------- phase D: FFN chunk pairs ------------------------
        with (
            tc.tile_pool(name="hp", bufs=1) as h_pool,
            tc.tile_pool(name="wdp", bufs=2) as wd_pool,
            tc.tile_pool(name="sgp", bufs=1) as sg_pool,
            tc.tile_pool(name="op", bufs=2) as o_pool,
            tc.tile_pool(name="psG", bufs=2, space="PSUM") as psG,
            tc.tile_pool(name="psU", bufs=2, space="PSUM") as psU,
            tc.tile_pool(name="psR", bufs=2, space="PSUM") as psR,
            tc.tile_pool(name="psS", bufs=2, space="PSUM") as psS,
        ):
            for pair in range(NCHUNK // 2):
                x_sbs, h_sbs, rs_ins = [], [], []
                for cc in range(2):
                    n = pair * 2 + cc
                    if n in x_tiles:
                        x_sb = x_tiles.pop(n)
                    else:
                        x_sb = x_pool.tile([128, KT, TC], f32r, name=f"x{n}", tag="x")
                        for k in range(KT):
                            nc.sync.dma_start(
                                out=x_sb[:, k, :],
                                in_=xT_ap[k * 128:(k + 1) * 128, n * TC:(n + 1) * TC],
                            )
                    x_sbs.append(x_sb)
                    h_sbs.append(h_pool.tile([128, MT, TC], f32r, name=f"h{n}", tag=f"h{cc}"))
                    rs_ins.append(dram.tile([TC, H], f32, name=f"rs_in{n}"))
                # stage 1: one weight pass covers both chunks of the pair
                for m in range(MT):
                    if (pair, m) in w_tiles:
                        w1b, w2b = w_tiles.pop((pair, m))
                    else:
                        w1b, w2b = fetch_wblk(pair, m)
                    for cc in range(2):
                        n = pair * 2 + cc
                        x_sb, h_sb = x_sbs[cc], h_sbs[cc]
                        psG_t = psG.tile([128, TC], f32, name=f"psG{n}_{m}", tag="psG")
                        for k in range(KT):
                            nc.tensor.matmul(
                                psG_t[:], w1b[:, k, :], x_sb[:, k, :],
                                start=(k == 0), stop=(k == KT - 1),
                            )
                        psU_t = psU.tile([128, TC], f32, name=f"psU{n}_{m}", tag="psU")
                        for k in range(KT):
                            nc.tensor.matmul(
                                psU_t[:], w2b[:, k, :], x_sb[:, k, :],
                                start=(k == 0), stop=(k == KT - 1),
                            )
                        sg = sg_pool.tile([128, TC], f32, name=f"sg{n}_{m}", tag=f"sg{cc}")
                        nc.scalar.activation(sg[:], psG_t[:], Act.Silu)
                        nc.vector.tensor_tensor(h_sb[:, m, :], sg[:], psU_t[:], op=Alu.mult)

                # stage 2. First pair: nh outer so one wd load serves both
                # chunks. Last pair: chunk-major with the later chunk first so
                # its ReduceScatter overlaps the earlier chunk's compute and
                # only one RS is exposed at the kernel tail.
                last = pair == NCHUNK // 2 - 1
                if last:
                    loop = [(cc, nh) for cc in (1, 0) for nh in range(NH)]
                else:
                    loop = [(cc, nh) for nh in range(NH) for cc in (0, 1)]
                rs_done = set()
                for cc, nh in loop:
                    if (not last and cc == 0) or (last and True):
                        wd_t = wd_pool.tile(
                            [128, MT, HC], f32r, name=f"wd{pair}_{cc}_{nh}", tag="wd"
                        )
                        nc.sync.dma_start(
                            out=wd_t[:], in_=w3_ap[nh].rearrange("p (k j) -> p k j", k=MT)
                        )
                    if True:
                        n = pair * 2 + cc
                        h_sb = h_sbs[cc]
                        for ts in range(TC // 128):
                            psR_t = psR.tile([128, HC], f32, name=f"psR{n}{nh}{ts}", tag="psR")
                            for kI in range(MR):
                                nc.tensor.matmul(
                                    psR_t[:], h_sb[:, kI, ts * 128:(ts + 1) * 128], wd_t[:, kI, :],
                                    start=(kI == 0), stop=(kI == MR - 1),
                                )
                            psS_t = psS.tile([128, HC], f32, name=f"psS{n}{nh}{ts}", tag="psS")
                            for kI in range(MR, MT):
                                nc.tensor.matmul(
                                    psS_t[:], h_sb[:, kI, ts * 128:(ts + 1) * 128], wd_t[:, kI, :],
                                    start=(kI == MR), stop=(kI == MT - 1),
                                )
                            g = n * (TC // 128) + ts
                            tr = o_pool.tile([128, HC], f32, name=f"tr{n}{nh}{ts}", tag="tr")
                            nc.scalar.activation(tr[:], psR_t[:], Act.Copy, scale=w_sb[:, g:g + 1])
                            ot = o_pool.tile([128, HC], f32, name=f"o{n}{nh}{ts}", tag="o")
                            nc.vector.scalar_tensor_tensor(
                                ot[:], psS_t[:], 1.0 / SH, tr[:], op0=Alu.mult, op1=Alu.add
                            )
                            nc.sync.dma_start(
                                out=rs_ins[cc][ts * 128:(ts + 1) * 128, nh * HC:(nh + 1) * HC],
                                in_=ot[:],
                            )
                    if (cc, nh) == loop[-1] or (last and nh == NH - 1):
                        if cc not in rs_done:
                            rs_done.add(cc)
                            nn_ = pair * 2 + cc
                            rs_out = dram.tile([TC // NC_, H], f32, name=f"rs_out{nn_}")
                            nc.gpsimd.collective_compute(
                                "ReduceScatter", Alu.add, replica_groups=RG,
                                ins=[rs_ins[cc].opt()], outs=[rs_out.opt()],
                            )
                            nc.sync.dma_start(out=out_ap[nn_], in_=rs_out[:])
                if not last:
                    for cc in (1, 0):
                        if cc not in rs_done:
                            rs_done.add(cc)
                            nn_ = pair * 2 + cc
                            rs_out = dram.tile([TC // NC_, H], f32, name=f"rs_out{nn_}")
                            nc.gpsimd.collective_compute(
                                "ReduceScatter", Alu.add, replica_groups=RG,
                                ins=[rs_ins[cc].opt()], outs=[rs_out.opt()],
                            )
                            nc.sync.dma_start(out=out_ap[nn_], in_=rs_out[:])

    nc.compile()
    return nc


def _get_prog():
    global _PROG
    if _PROG is None:
        _PROG = _build()
    return _PROG


def _prep_in_maps(hidden_states, gate_w, wg, wu, wd, swg, swu, swd):
    hs = np.ascontiguousarray(np.asarray(hidden_states, dtype=np.float32))
    gate_w = np.ascontiguousarray(np.asarray(gate_w, dtype=np.float32))
    wg = np.asarray(wg, dtype=np.float32)
    wu = np.asarray(wu, dtype=np.float32)
    wd = np.asarray(wd, dtype=np.float32)
    swg = np.asarray(swg, dtype=np.float32)
    swu = np.asarray(swu, dtype=np.float32)
    swd = np.asarray(swd, dtype=np.float32)

    X = hs.reshape(T, H)
    xT = np.ascontiguousarray(X.T)  # [H, T]
    in_maps = []
    for c in range(NC_):
        cs = slice(c * ISH, (c + 1) * ISH)
        w1 = np.concatenate([wg[c], swg[0][:, cs], swg[1][:, cs]], axis=1)
        w2 = np.concatenate([wu[c], swu[0][:, cs], swu[1][:, cs]], axis=1)
        w3 = np.concatenate([wd[c], swd[0][cs, :], swd[1][cs, :]], axis=0)
        # block for contiguous per-partition DMA lines (pure layout)
        w1 = np.ascontiguousarray(
            w1.reshape(KT, 128, MT, 128).transpose(2, 1, 0, 3).reshape(MT, 128, KT * 128))
        w2 = np.ascontiguousarray(
            w2.reshape(KT, 128, MT, 128).transpose(2, 1, 0, 3).reshape(MT, 128, KT * 128))
        w3 = np.ascontiguousarray(
            w3.reshape(MT, 128, NH, HC).transpose(2, 1, 0, 3).reshape(NH, 128, MT * HC))
        xsh = np.ascontiguousarray(xT[:, c * TSH:(c + 1) * TSH])
        esel = np.zeros((128, E), dtype=np.float32)
        esel[:, c] = 1.0
        in_maps.append({
            "xT": xT, "xsh": xsh, "gw": gate_w, "esel": esel,
            "w1": w1, "w2": w2, "w3": w3,
        })
    return in_maps


def _assemble(results):
    out = np.empty((T, H), dtype=np.float32)
    for c in range(NC_):
        sh = results[c]["out_sh"]  # [NCHUNK, TC//NC_, H]
        for n in range(NCHUNK):
            r0 = n * TC + c * (TC // NC_)
            out[r0:r0 + TC // NC_] = sh[n]
    loss = np.float32(results[0]["loss"][0, 0])
    return out.reshape(B, S, H), loss


def _run(inputs, trace=False, trace_cores=None):
    from concourse.bass_utils import run_bass_kernel_spmd

    nc = _get_prog()
    in_maps = _prep_in_maps(**inputs)
    last_exc = None
    for _attempt in range(3):
        try:
            res = run_bass_kernel_spmd(
                nc, in_maps, list(range(NC_)), trace=trace, trace_cores=trace_cores,
            )
            return _assemble(res.results), res
        except Exception as e:  # transient NRT device errors: retry
            last_exc = e
            if "UNAVAILABLE" not in str(e) and "UNRECOVERABLE" not in str(e):
                raise
    raise last_exc


def kernel(**inputs):
    (out, loss), _res = _run(inputs, trace=False)
    return out, loss


# revision 9
# speedup vs baseline: 1.0168x; 1.0168x over previous
"""MoE layer (8 routed experts top-2 + 2 shared experts) on 8 Trainium2 cores.

Strategy: expert-parallel, dense-masked.
  - Core c holds routed expert c's weights plus a 1/8 I-slice of both shared
    experts, concatenated into one augmented SwiGLU FFN of width 1280.
  - X is replicated (host sends X.T so every matmul is transpose-free:
    contraction dim H/I always lands on SBUF partitions).
  - Router: each core computes fp32 gate logits for its T/8 token shard on
    the PE, AllGather -> identical full logits everywhere; top-2 / softmax /
    combine weights / aux loss computed with DVE/ACT ops.
  - FFN matmuls run in float32r (full PE rate at N>=256, ~1.5e-4 rel err).
  - Tokens are processed in pairs of 512-token chunks; one pass over the
    w1/w2/w3 weights covers both chunks of a pair (halves weight traffic).
  - Routed output scaled per-token by the combine weight at PSUM drain
    (ACT Copy with per-partition scale); shared output added with a 0.5
    factor (SH=2 mean) in the same fused DVE op.
  - Output combined across cores with a per-512-token-chunk ReduceScatter
    (overlaps with compute); host reassembles the shards.

kernel(**inputs) -> (out [2,1024,2048] fp32, router_loss fp32 scalar),
matching reference.reference()'s return structure.
"""

import numpy as np

# ---- problem constants (hardcoded; kernel.py must be self-contained) ----
B, S, H, I = 2, 1024, 2048, 1024
E, K_TOP, SH = 8, 2, 2
AUX_COEF = 0.01
T = B * S                  # 2048 tokens
NC_ = 8                    # cores
ISH = I // NC_             # 128: per-core slice of each shared expert
IAUG = I + SH * ISH        # 1280 augmented intermediate width
TC = 512                   # token chunk
NCHUNK = T // TC           # 4
TSH = T // NC_             # 256: tokens per core shard (router + output)
KT = H // 128              # 16 contraction tiles
MT = IAUG // 128           # 10 I tiles
MR = I // 128              # 8 routed I tiles
HC = 256                   # H chunk in stage 2
NH = H // HC               # 8

_PROG = None  # cached compiled program


def _build():
    import concourse.bacc as bacc
    import concourse.mybir as mybir
    import concourse.tile as tile

    f32 = mybir.dt.float32
    f32r = mybir.dt.float32r
    Alu = mybir.AluOpType
    Act = mybir.ActivationFunctionType
    RG = [list(range(NC_))]

    nc = bacc.Bacc("TRN2", target_bir_lowering=False, debug=False, num_devices=NC_)

    xT_ap = nc.dram_tensor("xT", [H, T], f32r, kind="ExternalInput").ap()
    xsh_ap = nc.dram_tensor("xsh", [H, TSH], f32, kind="ExternalInput").ap()
    gw_ap = nc.dram_tensor("gw", [H, E], f32, kind="ExternalInput").ap()
    esel_ap = nc.dram_tensor("esel", [128, E], f32, kind="ExternalInput").ap()
    # w1/w2 blocked: [MT, 128p, KT*128] with [m, p, k*128+j] = w1[k*128+p, m*128+j]
    w1_ap = nc.dram_tensor("w1", [MT, 128, KT * 128], f32r, kind="ExternalInput").ap()
    w2_ap = nc.dram_tensor("w2", [MT, 128, KT * 128], f32r, kind="ExternalInput").ap()
    # w3 blocked: [NH, 128p, MT*HC] with [nh, p, kI*HC+j] = w3[kI*128+p, nh*HC+j]
    w3_ap = nc.dram_tensor("w3", [NH, 128, MT * HC], f32r, kind="ExternalInput").ap()
    out_ap = nc.dram_tensor("out_sh", [NCHUNK, TC // NC_, H], f32, kind="ExternalOutput").ap()
    loss_ap = nc.dram_tensor("loss", [1, 1], f32, kind="ExternalOutput").ap()

    TI = T // 128  # 16

    with tile.TileContext(nc) as tc:
      with (
        tc.tile_pool(name="sbr", bufs=1) as sbr,
        tc.tile_pool(name="dram", bufs=1, space="DRAM") as dram,
        tc.tile_pool(name="xp", bufs=2) as x_pool,
        tc.tile_pool(name="wblk", bufs=3) as wblk_pool,
      ):
        x_tiles = {}
        w_tiles = {}

        def fetch_wblk(pair, m):
            # 4 k-group DMAs per block so early matmuls only wait for their slice
            w1b = wblk_pool.tile([128, KT, 128], f32r, name=f"w1_{pair}_{m}", tag="w1")
            w2b = wblk_pool.tile([128, KT, 128], f32r, name=f"w2_{pair}_{m}", tag="w2")
            for g in range(4):
                ksl = slice(g * 4, (g + 1) * 4)
                csl = slice(g * 512, (g + 1) * 512)
                nc.sync.dma_start(
                    out=w1b[:, ksl, :],
                    in_=w1_ap[m][:, csl].rearrange("p (k j) -> p k j", k=4))
                nc.sync.dma_start(
                    out=w2b[:, ksl, :],
                    in_=w2_ap[m][:, csl].rearrange("p (k j) -> p k j", k=4))
            return w1b, w2b

        # ---------------- phase A: gate logits for own shard + AllGather --
        with (
            tc.tile_pool(name="xshp", bufs=1) as xsh_pool,
            tc.tile_pool(name="psLG", bufs=2, space="PSUM") as psLG,
            tc.tile_pool(name="psL", bufs=1, space="PSUM") as psL,
        ):
            xsh_sb = xsh_pool.tile([128, KT, TSH], f32, name="xsh_sb")
            for k in range(KT):
                nc.sync.dma_start(out=xsh_sb[:, k, :], in_=xsh_ap[k * 128:(k + 1) * 128, :])
            gw_sb = sbr.tile([128, KT, E], f32, name="gw_sb")
            for k in range(KT):
                nc.sync.dma_start(out=gw_sb[:, k, :], in_=gw_ap[k * 128:(k + 1) * 128, :])
            lgsh = sbr.tile([128, TSH // 128, E], f32, name="lgsh")
            for tt in range(TSH // 128):
                plg = psLG.tile([128, E], f32, name=f"plg{tt}", tag="plg")
                for k in range(KT):
                    nc.tensor.matmul(
                        plg[:], xsh_sb[:, k, tt * 128:(tt + 1) * 128], gw_sb[:, k, :],
                        start=(k == 0), stop=(k == KT - 1),
                    )
                nc.vector.tensor_copy(lgsh[:, tt, :], plg[:])
            ag_in = dram.tile([TSH, E], f32, name="ag_in")
            nc.sync.dma_start(
                out=ag_in[:].rearrange("(tt p) e -> p tt e", p=128), in_=lgsh[:]
            )
            lg_full = dram.tile([T, E], f32, name="lg_full", addr_space="Shared")
            nc.gpsimd.collective_compute(
                "AllGather", Alu.bypass, replica_groups=RG,
                ins=[ag_in.opt()], outs=[lg_full.opt()],
            )
            lg = sbr.tile([128, TI, E], f32, name="lg")
            nc.sync.dma_start(out=lg[:], in_=lg_full[:].rearrange("(ti p) e -> p ti e", p=128))

            # ------------- phase B: router math (identical on all cores) --
            def b3(ap2d):  # [128, TI] -> broadcast [128, TI, E]
                return ap2d.rearrange("p (t o) -> p t o", o=1).broadcast_to([128, TI, E])

            m1 = sbr.tile([128, TI], f32, name="m1")
            nc.vector.tensor_reduce(m1[:], lg[:], axis=mybir.AxisListType.X, op=Alu.max)
            mask = sbr.tile([128, TI, E], f32, name="mask")
            nc.vector.tensor_tensor(mask[:], lg[:], b3(m1[:]), op=Alu.is_equal)
            l2 = sbr.tile([128, TI, E], f32, name="l2")
            nc.vector.scalar_tensor_tensor(l2[:], mask[:], -1e30, lg[:], op0=Alu.mult, op1=Alu.add)
            m2 = sbr.tile([128, TI], f32, name="m2")
            nc.vector.tensor_reduce(m2[:], l2[:], axis=mybir.AxisListType.X, op=Alu.max)

            dlg = sbr.tile([128, TI, E], f32, name="dlg")
            nc.vector.tensor_tensor(dlg[:], lg[:], b3(m1[:]), op=Alu.subtract)
            pex = sbr.tile([128, TI, E], f32, name="pex")
            nc.scalar.activation(pex[:], dlg[:], Act.Exp)
            Z = sbr.tile([128, TI], f32, name="Z")
            nc.vector.tensor_reduce(Z[:], pex[:], axis=mybir.AxisListType.X, op=Alu.add)
            rZ = sbr.tile([128, TI], f32, name="rZ")
            nc.vector.reciprocal(rZ[:], Z[:])
            pn = sbr.tile([128, TI, E], f32, name="pn")
            nc.vector.tensor_tensor(pn[:], pex[:], b3(rZ[:]), op=Alu.mult)
            ind2 = sbr.tile([128, TI, E], f32, name="ind2")
            nc.vector.tensor_tensor(ind2[:], lg[:], b3(m2[:]), op=Alu.is_ge)

            sum_p = sbr.tile([128, E], f32, name="sum_p")
            nc.vector.tensor_reduce(
                sum_p[:], pn[:].rearrange("p t e -> p e t"),
                axis=mybir.AxisListType.X, op=Alu.add,
            )
            sum_i = sbr.tile([128, E], f32, name="sum_i")
            nc.vector.tensor_reduce(
                sum_i[:], ind2[:].rearrange("p t e -> p e t"),
                axis=mybir.AxisListType.X, op=Alu.add,
            )
            ones = sbr.tile([128, 1], f32, name="ones")
            nc.vector.memset(ones[:], 1.0)
            ptpe = psL.tile([1, E], f32, name="ptpe", tag="ptpe")
            nc.tensor.matmul(ptpe[:], ones[:], sum_i[:], start=True, stop=True)
            pppe = psL.tile([1, E], f32, name="pppe", tag="pppe")
            nc.tensor.matmul(pppe[:], ones[:], sum_p[:], start=True, stop=True)
            ppe_sb = sbr.tile([1, E], f32, name="ppe_sb")
            nc.vector.tensor_copy(ppe_sb[:], pppe[:])
            lt = sbr.tile([1, E], f32, name="lt")
            nc.vector.tensor_tensor(lt[:], ptpe[:], ppe_sb[:], op=Alu.mult)
            ls = sbr.tile([1, 1], f32, name="ls")
            nc.vector.tensor_reduce(ls[:], lt[:], axis=mybir.AxisListType.X, op=Alu.add)
            lsc = sbr.tile([1, 1], f32, name="lsc")
            nc.vector.tensor_scalar_mul(lsc[:], ls[:], float(E) * AUX_COEF / (T * T))
            nc.sync.dma_start(out=loss_ap[:], in_=lsc[:])

            # combine weight for this core's expert: w = exp(lc-m1)*1{lc>=m2}/(1+exp(m2-m1))
            esel_sb = sbr.tile([128, E], f32, name="esel_sb")
            nc.sync.dma_start(out=esel_sb[:], in_=esel_ap[:])
            eselb = esel_sb[:].rearrange("p (t e) -> p t e", t=1).broadcast_to([128, TI, E])
            lcq = sbr.tile([128, TI, E], f32, name="lcq")
            nc.vector.tensor_tensor(lcq[:], lg[:], eselb, op=Alu.mult)
            lc = sbr.tile([128, TI], f32, name="lc")
            nc.vector.tensor_reduce(lc[:], lcq[:], axis=mybir.AxisListType.X, op=Alu.add)
            d2 = sbr.tile([128, TI], f32, name="d2")
            nc.vector.tensor_tensor(d2[:], lc[:], m1[:], op=Alu.subtract)
            num = sbr.tile([128, TI], f32, name="num")
            nc.scalar.activation(num[:], d2[:], Act.Exp)
            d3 = sbr.tile([128, TI], f32, name="d3")
            nc.vector.tensor_tensor(d3[:], m2[:], m1[:], op=Alu.subtract)
            a2 = sbr.tile([128, TI], f32, name="a2")
            nc.scalar.activation(a2[:], d3[:], Act.Exp)
            den = sbr.tile([128, TI], f32, name="den")
            nc.vector.tensor_scalar_add(den[:], a2[:], 1.0)
            rden = sbr.tile([128, TI], f32, name="rden")
            nc.vector.reciprocal(rden[:], den[:])
            ind = sbr.tile([128, TI], f32, name="ind")
            nc.vector.tensor_tensor(ind[:], lc[:], m2[:], op=Alu.is_ge)
            wA = sbr.tile([128, TI], f32, name="wA")
            nc.vector.tensor_tensor(wA[:], num[:], ind[:], op=Alu.mult)
            w_sb = sbr.tile([128, TI], f32, name="w_sb")
            nc.vector.tensor_tensor(w_sb[:], wA[:], rden[:], op=Alu.mult)

        # ---------------- phase D: FFN chunk pairs ------------------------
        with (
            tc.tile_pool(name="hp", bufs=1) as h_pool,
            tc.tile_pool(name="wdp", bufs=2) as wd_pool,
            tc.tile_pool(name="sgp", bufs=1) as sg_pool,
            tc.tile_pool(name="op", bufs=2) as o_pool,
            tc.tile_pool(name="psG", bufs=2, space="PSUM") as psG,
            tc.tile_pool(name="psU", bufs=2, space="PSUM") as psU,
            tc.tile_pool(name="psR", bufs=2, space="PSUM") as psR,
            tc.tile_pool(name="psS", bufs=2, space="PSUM") as psS,
        ):
            for pair in range(NCHUNK // 2):
                x_sbs, h_sbs, rs_ins = [], [], []
                for cc in range(2):
                    n = pair * 2 + cc
                    if n in x_tiles:
                        x_sb = x_tiles.pop(n)
                    else:
                        x_sb = x_pool.tile([128, KT, TC], f32r, name=f"x{n}", tag="x")
                        for k in range(KT):
                            nc.sync.dma_start(
                                out=x_sb[:, k, :],
                                in_=xT_ap[k * 128:(k + 1) * 128, n * TC:(n + 1) * TC],
                            )
                    x_sbs.append(x_sb)
                    h_sbs.append(h_pool.tile([128, MT, TC], f32r, name=f"h{n}", tag=f"h{cc}"))
                    rs_ins.append(dram.tile([TC, H], f32, name=f"rs_in{n}"))
                # stage 1: one weight pass covers both chunks of the pair
                for m in range(MT):
                    if (pair, m) in w_tiles:
                        w1b, w2b = w_tiles.pop((pair, m))
                    else:
                        w1b, w2b = fetch_wblk(pair, m)
                    for cc in range(2):
                        n = pair * 2 + cc
                        x_sb, h_sb = x_sbs[cc], h_sbs[cc]
                        psG_t = psG.tile([128, TC], f32, name=f"psG{n}_{m}", tag="psG")
                        for k in range(KT):
                            nc.tensor.matmul(
                                psG_t[:], w1b[:, k, :], x_sb[:, k, :],
                                start=(k == 0), stop=(k == KT - 1),
                            )
                        psU_t = psU.tile([128, TC], f32, name=f"psU{n}_{m}", tag="psU")
                        for k in range(KT):
                            nc.tensor.matmul(
                                psU_t[:], w2b[:, k, :], x_sb[:, k, :],
                                start=(k == 0), stop=(k == KT - 1),
                            )
                        sg = sg_pool.tile([128, TC], f32, name=f"sg{n}_{m}", tag=f"sg{cc}")
                        nc.scalar.activation(sg[:], psG_t[:], Act.Silu)
                        nc.vector.tensor_tensor(h_sb[:, m, :], sg[:], psU_t[:], op=Alu.mult)

                # stage 2. First pair: nh outer so one wd load serves both
                # chunks. Last pair: chunk-major with the later chunk first so
                # its ReduceScatter overlaps the earlier chunk's compute and
                # only one RS is exposed at the kernel tail.
                last = pair == NCHUNK // 2 - 1
                if last:
                    loop = [(cc, nh) for cc in (1, 0) for nh in range(NH)]
                else:
                    loop = [(cc, nh) for nh in range(NH) for cc in (0, 1)]
                rs_done = set()
                for cc, nh in loop:
                    if (not last and cc == 0) or (last and True):
                        wd_t = wd_pool.tile(
                            [128, MT, HC], f32r, name=f"wd{pair}_{cc}_{nh}", tag="wd"
                        )
                        nc.sync.dma_start(
                            out=wd_t[:], in_=w3_ap[nh].rearrange("p (k j) -> p k j", k=MT)
                        )
                    if True:
                        n = pair * 2 + cc
                        h_sb = h_sbs[cc]
                        for ts in range(TC // 128):
                            psR_t = psR.tile([128, HC], f32, name=f"psR{n}{nh}{ts}", tag="psR")
                            for kI in range(MR):
                                nc.tensor.matmul(
                                    psR_t[:], h_sb[:, kI, ts * 128:(ts + 1) * 128], wd_t[:, kI, :],
                                    start=(kI == 0), stop=(kI == MR - 1),
                                )
                            psS_t = psS.tile([128, HC], f32, name=f"psS{n}{nh}{ts}", tag="psS")
                            for kI in range(MR, MT):
                                nc.tensor.matmul(
                                    psS_t[:], h_sb[:, kI, ts * 128:(ts + 1) * 128], wd_t[:, kI, :],
                                    start=(kI == MR), stop=(kI == MT - 1),
                                )
                            g = n * (TC // 128) + ts
                            tr = o_pool.tile([128, HC], f32, name=f"tr{n}{nh}{ts}", tag="tr")
                            nc.scalar.activation(tr[:], psR_t[:], Act.Copy, scale=w_sb[:, g:g + 1])
                            ot = o_pool.tile([128, HC], f32, name=f"o{n}{nh}{ts}", tag="o")
                            nc.vector.scalar_tensor_tensor(
                                ot[:], psS_t[:], 1.0 / SH, tr[:], op0=Alu.mult, op1=Alu.add
                            )
                            nc.sync.dma_start(
                                out=rs_ins[cc][ts * 128:(ts + 1) * 128, nh * HC:(nh + 1) * HC],
                                in_=ot[:],
                            )
                    if (cc, nh) == loop[-1] or (last and nh == NH - 1):
                        if cc not in rs_done:
                            rs_done.add(cc)
                            nn_ = pair * 2 + cc
                            rs_out = dram.tile([TC // NC_, H], f32, name=f"rs_out{nn_}")
                            nc.gpsimd.collective_compute(
                                "ReduceScatter", Alu.add, replica_groups=RG,
                                ins=[rs_ins[cc].opt()], outs=[rs_out.opt()],
                            )
                            nc.sync.dma_start(out=out_ap[nn_], in_=rs_out[:])
                if not last:
                    for cc in (1, 0):
                        if cc not in rs_done:
                            rs_done.add(cc)
                            nn_ = pair * 2 + cc
                            rs_out = dram.tile([TC // NC_, H], f32, name=f"rs_out{nn_}")
                            nc.gpsimd.collective_compute(
                                "ReduceScatter", Alu.add, replica_groups=RG,
                                ins=[rs_ins[cc].opt()], outs=[rs_out.opt()],
                            )
                            nc.sync.dma_start(out=out_ap[nn_], in_=rs_out[:])

    nc.compile()
    return nc


def _get_prog():
    global _PROG
    if _PROG is None:
        _PROG = _build()
    return _PROG


def _prep_in_maps(hidden_states, gate_w, wg, wu, wd, swg, swu, swd):
    hs = np.ascontiguousarray(np.asarray(hidden_states, dtype=np.float32))
    gate_w = np.ascontiguousarray(np.asarray(gate_w, dtype=np.float32))
    wg = np.asarray(wg, dtype=np.float32)
    wu = np.asarray(wu, dtype=np.float32)
    wd = np.asarray(wd, dtype=np.float32)
    swg = np.asarray(swg, dtype=np.float32)
    swu = np.asarray(swu, dtype=np.float32)
    swd = np.asarray(swd, dtype=np.float32)

    X = hs.reshape(T, H)
    xT = np.ascontiguousarray(X.T)  # [H, T]
    in_maps = []
    for c in range(NC_):
        cs = slice(c * ISH, (c + 1) * ISH)
        w1 = np.concatenate([wg[c], swg[0][:, cs], swg[1][:, cs]], axis=1)
        w2 = np.concatenate([wu[c], swu[0][:, cs], swu[1][:, cs]], axis=1)
        w3 = np.concatenate([wd[c], swd[0][cs, :], swd[1][cs, :]], axis=0)
        # block for contiguous per-partition DMA lines (pure layout)
        w1 = np.ascontiguousarray(
            w1.reshape(KT, 128, MT, 128).transpose(2, 1, 0, 3).reshape(MT, 128, KT * 128))
        w2 = np.ascontiguousarray(
            w2.reshape(KT, 128, MT, 128).transpose(2, 1, 0, 3).reshape(MT, 128, KT * 128))
        w3 = np.ascontiguousarray(
            w3.reshape(MT, 128, NH, HC).transpose(2, 1, 0, 3).reshape(NH, 128, MT * HC))
        xsh = np.ascontiguousarray(xT[:, c * TSH:(c + 1) * TSH])
        esel = np.zeros((128, E), dtype=np.float32)
        esel[:, c] = 1.0
        in_maps.append({
            "xT": xT, "xsh": xsh, "gw": gate_w, "esel": esel,
            "w1": w1, "w2": w2, "w3": w3,
        })
    return in_maps


def _assemble(results):
    out = np.empty((T, H), dtype=np.float32)
    for c in range(NC_):
        sh = results[c]["out_sh"]  # [NCHUNK, TC//NC_, H]
        for n in range(NCHUNK):
            r0 = n * TC + c * (TC // NC_)
            out[r0:r0 + TC // NC_] = sh[n]
    loss = np.float32(results[0]["loss"][0, 0])
    return out.reshape(B, S, H), loss


def _run(inputs, trace=False, trace_cores=None):
    from concourse.bass_utils import run_bass_kernel_spmd

    nc = _get_prog()
    in_maps = _prep_in_maps(**inputs)
    last_exc = None
    for _attempt in range(3):
        try:
            res = run_bass_kernel_spmd(
                nc, in_maps, list(range(NC_)), trace=trace, trace_cores=trace_cores,
            )
            return _assemble(res.results), res
        except Exception as e:  # transient NRT device errors: retry
            last_exc = e
            if "UNAVAILABLE" not in str(e) and "UNRECOVERABLE" not in str(e):
                raise
    raise last_exc


def kernel(**inputs):
    (out, loss), _res = _run(inputs, trace=False)
    return out, loss


# revision 10
# speedup vs baseline: 1.1594x; 1.1402x over previous
"""MoE layer (8 routed experts top-2 + 2 shared experts) on 8 Trainium2 cores.

Strategy: expert-parallel, dense-masked.
  - Core c holds routed expert c's weights plus a 1/8 I-slice of both shared
    experts, concatenated into one augmented SwiGLU FFN of width 1280.
  - X is replicated (host sends X.T so every matmul is transpose-free:
    contraction dim H/I always lands on SBUF partitions).
  - Router: each core computes fp32 gate logits for its T/8 token shard on
    the PE, AllGather -> identical full logits everywhere; top-2 / softmax /
    combine weights / aux loss computed with DVE/ACT ops.
  - FFN matmuls run in float32r (full PE rate at N>=256, ~1.5e-4 rel err).
  - Tokens are processed in pairs of 512-token chunks; one pass over the
    w1/w2/w3 weights covers both chunks of a pair (halves weight traffic).
  - Routed output scaled per-token by the combine weight at PSUM drain
    (ACT Copy with per-partition scale); shared output added with a 0.5
    factor (SH=2 mean) in the same fused DVE op.
  - Output combined across cores with a per-512-token-chunk ReduceScatter
    (overlaps with compute); host reassembles the shards.

kernel(**inputs) -> (out [2,1024,2048] fp32, router_loss fp32 scalar),
matching reference.reference()'s return structure.
"""

import numpy as np

# ---- problem constants (hardcoded; kernel.py must be self-contained) ----
B, S, H, I = 2, 1024, 2048, 1024
E, K_TOP, SH = 8, 2, 2
AUX_COEF = 0.01
T = B * S                  # 2048 tokens
NC_ = 8                    # cores
ISH = I // NC_             # 128: per-core slice of each shared expert
IAUG = I + SH * ISH        # 1280 augmented intermediate width
TC = 512                   # token chunk
NCHUNK = T // TC           # 4
TSH = T // NC_             # 256: tokens per core shard (router + output)
KT = H // 128              # 16 contraction tiles
MT = IAUG // 128           # 10 I tiles
MR = I // 128              # 8 routed I tiles
BF16 = True                # matmul operand precision: bf16 (fast) vs float32r
HC = 512 if BF16 else 256  # H chunk in stage 2
NH = H // HC

_PROG = None  # cached compiled program


def _build():
    import concourse.bacc as bacc
    import concourse.mybir as mybir
    import concourse.tile as tile

    f32 = mybir.dt.float32
    f32r = mybir.dt.bfloat16 if BF16 else mybir.dt.float32r
    Alu = mybir.AluOpType
    Act = mybir.ActivationFunctionType
    RG = [list(range(NC_))]

    nc = bacc.Bacc("TRN2", target_bir_lowering=False, debug=False, num_devices=NC_)

    xT_ap = nc.dram_tensor("xT", [H, T], f32r, kind="ExternalInput").ap()
    xsh_ap = nc.dram_tensor("xsh", [H, TSH], f32, kind="ExternalInput").ap()
    gw_ap = nc.dram_tensor("gw", [H, E], f32, kind="ExternalInput").ap()
    esel_ap = nc.dram_tensor("esel", [128, E], f32, kind="ExternalInput").ap()
    # w1/w2 blocked: [MT, 128p, KT*128] with [m, p, k*128+j] = w1[k*128+p, m*128+j]
    w1_ap = nc.dram_tensor("w1", [MT, 128, KT * 128], f32r, kind="ExternalInput").ap()
    w2_ap = nc.dram_tensor("w2", [MT, 128, KT * 128], f32r, kind="ExternalInput").ap()
    # w3 blocked: [NH, 128p, MT*HC] with [nh, p, kI*HC+j] = w3[kI*128+p, nh*HC+j]
    w3_ap = nc.dram_tensor("w3", [NH, 128, MT * HC], f32r, kind="ExternalInput").ap()
    out_ap = nc.dram_tensor("out_sh", [NCHUNK, TC // NC_, H], f32, kind="ExternalOutput").ap()
    loss_ap = nc.dram_tensor("loss", [1, 1], f32, kind="ExternalOutput").ap()

    TI = T // 128  # 16

    with tile.TileContext(nc) as tc:
      with (
        tc.tile_pool(name="sbr", bufs=1) as sbr,
        tc.tile_pool(name="dram", bufs=1, space="DRAM") as dram,
        tc.tile_pool(name="xp", bufs=2) as x_pool,
        tc.tile_pool(name="wblk", bufs=3) as wblk_pool,
      ):
        x_tiles = {}
        w_tiles = {}

        def fetch_wblk(pair, m):
            # 4 k-group DMAs per block so early matmuls only wait for their slice
            w1b = wblk_pool.tile([128, KT, 128], f32r, name=f"w1_{pair}_{m}", tag="w1")
            w2b = wblk_pool.tile([128, KT, 128], f32r, name=f"w2_{pair}_{m}", tag="w2")
            for g in range(4):
                ksl = slice(g * 4, (g + 1) * 4)
                csl = slice(g * 512, (g + 1) * 512)
                nc.sync.dma_start(
                    out=w1b[:, ksl, :],
                    in_=w1_ap[m][:, csl].rearrange("p (k j) -> p k j", k=4))
                nc.sync.dma_start(
                    out=w2b[:, ksl, :],
                    in_=w2_ap[m][:, csl].rearrange("p (k j) -> p k j", k=4))
            return w1b, w2b

        # ---------------- phase A: gate logits for own shard + AllGather --
        with (
            tc.tile_pool(name="xshp", bufs=1) as xsh_pool,
            tc.tile_pool(name="psLG", bufs=2, space="PSUM") as psLG,
            tc.tile_pool(name="psL", bufs=1, space="PSUM") as psL,
        ):
            xsh_sb = xsh_pool.tile([128, KT, TSH], f32, name="xsh_sb")
            for k in range(KT):
                nc.sync.dma_start(out=xsh_sb[:, k, :], in_=xsh_ap[k * 128:(k + 1) * 128, :])
            gw_sb = sbr.tile([128, KT, E], f32, name="gw_sb")
            for k in range(KT):
                nc.sync.dma_start(out=gw_sb[:, k, :], in_=gw_ap[k * 128:(k + 1) * 128, :])
            lgsh = sbr.tile([128, TSH // 128, E], f32, name="lgsh")
            for tt in range(TSH // 128):
                plg = psLG.tile([128, E], f32, name=f"plg{tt}", tag="plg")
                for k in range(KT):
                    nc.tensor.matmul(
                        plg[:], xsh_sb[:, k, tt * 128:(tt + 1) * 128], gw_sb[:, k, :],
                        start=(k == 0), stop=(k == KT - 1),
                    )
                nc.vector.tensor_copy(lgsh[:, tt, :], plg[:])
            ag_in = dram.tile([TSH, E], f32, name="ag_in")
            nc.sync.dma_start(
                out=ag_in[:].rearrange("(tt p) e -> p tt e", p=128), in_=lgsh[:]
            )
            lg_full = dram.tile([T, E], f32, name="lg_full", addr_space="Shared")
            nc.gpsimd.collective_compute(
                "AllGather", Alu.bypass, replica_groups=RG,
                ins=[ag_in.opt()], outs=[lg_full.opt()],
            )
            lg = sbr.tile([128, TI, E], f32, name="lg")
            nc.sync.dma_start(out=lg[:], in_=lg_full[:].rearrange("(ti p) e -> p ti e", p=128))

            # ------------- phase B: router math (identical on all cores) --
            def b3(ap2d):  # [128, TI] -> broadcast [128, TI, E]
                return ap2d.rearrange("p (t o) -> p t o", o=1).broadcast_to([128, TI, E])

            m1 = sbr.tile([128, TI], f32, name="m1")
            nc.vector.tensor_reduce(m1[:], lg[:], axis=mybir.AxisListType.X, op=Alu.max)
            mask = sbr.tile([128, TI, E], f32, name="mask")
            nc.vector.tensor_tensor(mask[:], lg[:], b3(m1[:]), op=Alu.is_equal)
            l2 = sbr.tile([128, TI, E], f32, name="l2")
            nc.vector.scalar_tensor_tensor(l2[:], mask[:], -1e30, lg[:], op0=Alu.mult, op1=Alu.add)
            m2 = sbr.tile([128, TI], f32, name="m2")
            nc.vector.tensor_reduce(m2[:], l2[:], axis=mybir.AxisListType.X, op=Alu.max)

            dlg = sbr.tile([128, TI, E], f32, name="dlg")
            nc.vector.tensor_tensor(dlg[:], lg[:], b3(m1[:]), op=Alu.subtract)
            pex = sbr.tile([128, TI, E], f32, name="pex")
            nc.scalar.activation(pex[:], dlg[:], Act.Exp)
            Z = sbr.tile([128, TI], f32, name="Z")
            nc.vector.tensor_reduce(Z[:], pex[:], axis=mybir.AxisListType.X, op=Alu.add)
            rZ = sbr.tile([128, TI], f32, name="rZ")
            nc.vector.reciprocal(rZ[:], Z[:])
            pn = sbr.tile([128, TI, E], f32, name="pn")
            nc.vector.tensor_tensor(pn[:], pex[:], b3(rZ[:]), op=Alu.mult)
            ind2 = sbr.tile([128, TI, E], f32, name="ind2")
            nc.vector.tensor_tensor(ind2[:], lg[:], b3(m2[:]), op=Alu.is_ge)

            sum_p = sbr.tile([128, E], f32, name="sum_p")
            nc.vector.tensor_reduce(
                sum_p[:], pn[:].rearrange("p t e -> p e t"),
                axis=mybir.AxisListType.X, op=Alu.add,
            )
            sum_i = sbr.tile([128, E], f32, name="sum_i")
            nc.vector.tensor_reduce(
                sum_i[:], ind2[:].rearrange("p t e -> p e t"),
                axis=mybir.AxisListType.X, op=Alu.add,
            )
            ones = sbr.tile([128, 1], f32, name="ones")
            nc.vector.memset(ones[:], 1.0)
            ptpe = psL.tile([1, E], f32, name="ptpe", tag="ptpe")
            nc.tensor.matmul(ptpe[:], ones[:], sum_i[:], start=True, stop=True)
            pppe = psL.tile([1, E], f32, name="pppe", tag="pppe")
            nc.tensor.matmul(pppe[:], ones[:], sum_p[:], start=True, stop=True)
            ppe_sb = sbr.tile([1, E], f32, name="ppe_sb")
            nc.vector.tensor_copy(ppe_sb[:], pppe[:])
            lt = sbr.tile([1, E], f32, name="lt")
            nc.vector.tensor_tensor(lt[:], ptpe[:], ppe_sb[:], op=Alu.mult)
            ls = sbr.tile([1, 1], f32, name="ls")
            nc.vector.tensor_reduce(ls[:], lt[:], axis=mybir.AxisListType.X, op=Alu.add)
            lsc = sbr.tile([1, 1], f32, name="lsc")
            nc.vector.tensor_scalar_mul(lsc[:], ls[:], float(E) * AUX_COEF / (T * T))
            nc.sync.dma_start(out=loss_ap[:], in_=lsc[:])

            # combine weight for this core's expert: w = exp(lc-m1)*1{lc>=m2}/(1+exp(m2-m1))
            esel_sb = sbr.tile([128, E], f32, name="esel_sb")
            nc.sync.dma_start(out=esel_sb[:], in_=esel_ap[:])
            eselb = esel_sb[:].rearrange("p (t e) -> p t e", t=1).broadcast_to([128, TI, E])
            lcq = sbr.tile([128, TI, E], f32, name="lcq")
            nc.vector.tensor_tensor(lcq[:], lg[:], eselb, op=Alu.mult)
            lc = sbr.tile([128, TI], f32, name="lc")
            nc.vector.tensor_reduce(lc[:], lcq[:], axis=mybir.AxisListType.X, op=Alu.add)
            d2 = sbr.tile([128, TI], f32, name="d2")
            nc.vector.tensor_tensor(d2[:], lc[:], m1[:], op=Alu.subtract)
            num = sbr.tile([128, TI], f32, name="num")
            nc.scalar.activation(num[:], d2[:], Act.Exp)
            d3 = sbr.tile([128, TI], f32, name="d3")
            nc.vector.tensor_tensor(d3[:], m2[:], m1[:], op=Alu.subtract)
            a2 = sbr.tile([128, TI], f32, name="a2")
            nc.scalar.activation(a2[:], d3[:], Act.Exp)
            den = sbr.tile([128, TI], f32, name="den")
            nc.vector.tensor_scalar_add(den[:], a2[:], 1.0)
            rden = sbr.tile([128, TI], f32, name="rden")
            nc.vector.reciprocal(rden[:], den[:])
            ind = sbr.tile([128, TI], f32, name="ind")
            nc.vector.tensor_tensor(ind[:], lc[:], m2[:], op=Alu.is_ge)
            wA = sbr.tile([128, TI], f32, name="wA")
            nc.vector.tensor_tensor(wA[:], num[:], ind[:], op=Alu.mult)
            w_sb = sbr.tile([128, TI], f32, name="w_sb")
            nc.vector.tensor_tensor(w_sb[:], wA[:], rden[:], op=Alu.mult)

        # ---------------- phase D: FFN chunk pairs ------------------------
        with (
            tc.tile_pool(name="hp", bufs=1) as h_pool,
            tc.tile_pool(name="wdp", bufs=2) as wd_pool,
            tc.tile_pool(name="sgp", bufs=1) as sg_pool,
            tc.tile_pool(name="op", bufs=2) as o_pool,
            tc.tile_pool(name="psG", bufs=2, space="PSUM") as psG,
            tc.tile_pool(name="psU", bufs=2, space="PSUM") as psU,
            tc.tile_pool(name="psR", bufs=2, space="PSUM") as psR,
            tc.tile_pool(name="psS", bufs=2, space="PSUM") as psS,
        ):
            for pair in range(NCHUNK // 2):
                x_sbs, h_sbs, rs_ins = [], [], []
                for cc in range(2):
                    n = pair * 2 + cc
                    if n in x_tiles:
                        x_sb = x_tiles.pop(n)
                    else:
                        x_sb = x_pool.tile([128, KT, TC], f32r, name=f"x{n}", tag="x")
                        for k in range(KT):
                            nc.sync.dma_start(
                                out=x_sb[:, k, :],
                                in_=xT_ap[k * 128:(k + 1) * 128, n * TC:(n + 1) * TC],
                            )
                    x_sbs.append(x_sb)
                    h_sbs.append(h_pool.tile([128, MT, TC], f32r, name=f"h{n}", tag=f"h{cc}"))
                    rs_ins.append(dram.tile([TC, H], f32, name=f"rs_in{n}"))
                # stage 1: one weight pass covers both chunks of the pair
                for m in range(MT):
                    if (pair, m) in w_tiles:
                        w1b, w2b = w_tiles.pop((pair, m))
                    else:
                        w1b, w2b = fetch_wblk(pair, m)
                    for cc in range(2):
                        n = pair * 2 + cc
                        x_sb, h_sb = x_sbs[cc], h_sbs[cc]
                        psG_t = psG.tile([128, TC], f32, name=f"psG{n}_{m}", tag="psG")
                        for k in range(KT):
                            nc.tensor.matmul(
                                psG_t[:], w1b[:, k, :], x_sb[:, k, :],
                                start=(k == 0), stop=(k == KT - 1),
                            )
                        psU_t = psU.tile([128, TC], f32, name=f"psU{n}_{m}", tag="psU")
                        for k in range(KT):
                            nc.tensor.matmul(
                                psU_t[:], w2b[:, k, :], x_sb[:, k, :],
                                start=(k == 0), stop=(k == KT - 1),
                            )
                        sg = sg_pool.tile([128, TC], f32, name=f"sg{n}_{m}", tag=f"sg{cc}")
                        nc.scalar.activation(sg[:], psG_t[:], Act.Silu)
                        nc.vector.tensor_tensor(h_sb[:, m, :], sg[:], psU_t[:], op=Alu.mult)

                # stage 2. First pair: nh outer so one wd load serves both
                # chunks. Last pair: chunk-major with the later chunk first so
                # its ReduceScatter overlaps the earlier chunk's compute and
                # only one RS is exposed at the kernel tail.
                last = pair == NCHUNK // 2 - 1
                if last:
                    loop = [(cc, nh) for cc in (1, 0) for nh in range(NH)]
                else:
                    loop = [(cc, nh) for nh in range(NH) for cc in (0, 1)]
                rs_done = set()
                for cc, nh in loop:
                    if (not last and cc == 0) or (last and True):
                        wd_t = wd_pool.tile(
                            [128, MT, HC], f32r, name=f"wd{pair}_{cc}_{nh}", tag="wd"
                        )
                        nc.sync.dma_start(
                            out=wd_t[:], in_=w3_ap[nh].rearrange("p (k j) -> p k j", k=MT)
                        )
                    if True:
                        n = pair * 2 + cc
                        h_sb = h_sbs[cc]
                        for ts in range(TC // 128):
                            psR_t = psR.tile([128, HC], f32, name=f"psR{n}{nh}{ts}", tag="psR")
                            for kI in range(MR):
                                nc.tensor.matmul(
                                    psR_t[:], h_sb[:, kI, ts * 128:(ts + 1) * 128], wd_t[:, kI, :],
                                    start=(kI == 0), stop=(kI == MR - 1),
                                )
                            psS_t = psS.tile([128, HC], f32, name=f"psS{n}{nh}{ts}", tag="psS")
                            for kI in range(MR, MT):
                                nc.tensor.matmul(
                                    psS_t[:], h_sb[:, kI, ts * 128:(ts + 1) * 128], wd_t[:, kI, :],
                                    start=(kI == MR), stop=(kI == MT - 1),
                                )
                            g = n * (TC // 128) + ts
                            tr = o_pool.tile([128, HC], f32, name=f"tr{n}{nh}{ts}", tag="tr")
                            nc.scalar.activation(tr[:], psR_t[:], Act.Copy, scale=w_sb[:, g:g + 1])
                            ot = o_pool.tile([128, HC], f32, name=f"o{n}{nh}{ts}", tag="o")
                            nc.vector.scalar_tensor_tensor(
                                ot[:], psS_t[:], 1.0 / SH, tr[:], op0=Alu.mult, op1=Alu.add
                            )
                            nc.sync.dma_start(
                                out=rs_ins[cc][ts * 128:(ts + 1) * 128, nh * HC:(nh + 1) * HC],
                                in_=ot[:],
                            )
                    if (cc, nh) == loop[-1] or (last and nh == NH - 1):
                        if cc not in rs_done:
                            rs_done.add(cc)
                            nn_ = pair * 2 + cc
                            rs_out = dram.tile([TC // NC_, H], f32, name=f"rs_out{nn_}")
                            nc.gpsimd.collective_compute(
                                "ReduceScatter", Alu.add, replica_groups=RG,
                                ins=[rs_ins[cc].opt()], outs=[rs_out.opt()],
                            )
                            nc.sync.dma_start(out=out_ap[nn_], in_=rs_out[:])
                if not last:
                    for cc in (1, 0):
                        if cc not in rs_done:
                            rs_done.add(cc)
                            nn_ = pair * 2 + cc
                            rs_out = dram.tile([TC // NC_, H], f32, name=f"rs_out{nn_}")
                            nc.gpsimd.collective_compute(
                                "ReduceScatter", Alu.add, replica_groups=RG,
                                ins=[rs_ins[cc].opt()], outs=[rs_out.opt()],
                            )
                            nc.sync.dma_start(out=out_ap[nn_], in_=rs_out[:])

    nc.compile()
    return nc


def _get_prog():
    global _PROG
    if _PROG is None:
        _PROG = _build()
    return _PROG


def _prep_in_maps(hidden_states, gate_w, wg, wu, wd, swg, swu, swd):
    hs = np.ascontiguousarray(np.asarray(hidden_states, dtype=np.float32))
    gate_w = np.ascontiguousarray(np.asarray(gate_w, dtype=np.float32))
    wg = np.asarray(wg, dtype=np.float32)
    wu = np.asarray(wu, dtype=np.float32)
    wd = np.asarray(wd, dtype=np.float32)
    swg = np.asarray(swg, dtype=np.float32)
    swu = np.asarray(swu, dtype=np.float32)
    swd = np.asarray(swd, dtype=np.float32)

    X = hs.reshape(T, H)
    xT = np.ascontiguousarray(X.T)  # [H, T]
    in_maps = []
    for c in range(NC_):
        cs = slice(c * ISH, (c + 1) * ISH)
        w1 = np.concatenate([wg[c], swg[0][:, cs], swg[1][:, cs]], axis=1)
        w2 = np.concatenate([wu[c], swu[0][:, cs], swu[1][:, cs]], axis=1)
        w3 = np.concatenate([wd[c], swd[0][cs, :], swd[1][cs, :]], axis=0)
        # block for contiguous per-partition DMA lines (pure layout)
        w1 = np.ascontiguousarray(
            w1.reshape(KT, 128, MT, 128).transpose(2, 1, 0, 3).reshape(MT, 128, KT * 128))
        w2 = np.ascontiguousarray(
            w2.reshape(KT, 128, MT, 128).transpose(2, 1, 0, 3).reshape(MT, 128, KT * 128))
        w3 = np.ascontiguousarray(
            w3.reshape(MT, 128, NH, HC).transpose(2, 1, 0, 3).reshape(NH, 128, MT * HC))
        xsh = np.ascontiguousarray(xT[:, c * TSH:(c + 1) * TSH])
        esel = np.zeros((128, E), dtype=np.float32)
        esel[:, c] = 1.0
        if BF16:
            import ml_dtypes
            bf = ml_dtypes.bfloat16
            w1, w2, w3 = w1.astype(bf), w2.astype(bf), w3.astype(bf)
            xT_in = xT.astype(bf)
        else:
            xT_in = xT
        in_maps.append({
            "xT": xT_in, "xsh": xsh, "gw": gate_w, "esel": esel,
            "w1": w1, "w2": w2, "w3": w3,
        })
    return in_maps


def _assemble(results):
    out = np.empty((T, H), dtype=np.float32)
    for c in range(NC_):
        sh = results[c]["out_sh"]  # [NCHUNK, TC//NC_, H]
        for n in range(NCHUNK):
            r0 = n * TC + c * (TC // NC_)
            out[r0:r0 + TC // NC_] = sh[n]
    loss = np.float32(results[0]["loss"][0, 0])
    return out.reshape(B, S, H), loss


def _run(inputs, trace=False, trace_cores=None):
    from concourse.bass_utils import run_bass_kernel_spmd

    nc = _get_prog()
    in_maps = _prep_in_maps(**inputs)
    last_exc = None
    for _attempt in range(3):
        try:
            res = run_bass_kernel_spmd(
                nc, in_maps, list(range(NC_)), trace=trace, trace_cores=trace_cores,
            )
            return _assemble(res.results), res
        except Exception as e:  # transient NRT device errors: retry
            last_exc = e
            if "UNAVAILABLE" not in str(e) and "UNRECOVERABLE" not in str(e):
                raise
    raise last_exc


def kernel(**inputs):
    (out, loss), _res = _run(inputs, trace=False)
    return out, loss
